# revision 6
# baseline (speedup 1.0000x reference)
"""Trainium2 Bass kernel for nn_ContrativeNet (gnn_message_passing).

Strategy: data-parallel over 8 NeuronCores, 64 contiguous 100-node graphs per
core. All GCN convolutions become dense per-graph [100,100] propagation
matmuls (uniform degree: every node has exactly DEG in-edges + self-loop for
the static graph, K for the dynamic knn graph, so the symmetric normalization
is a constant scale folded into the activation). The dilated knn is computed
on-device with the DVE top-8 instruction: S = G - sq[m]/2 is row-ranking
equivalent to -dist, self is always rank 0, and the dilated selection is
ranks {1,3,5,7}; the 0/1 adjacency is built by comparing S against the
selected top-8 values, then transposed on the PE.

Activations are kept feature-major ("T" layout, [F, nodes]) between layers so
the feature matmul (contracting Cin on partitions) needs no transposes, and
propagation matmuls use the U->T form (lhsT = z, rhs = M1^T) so their output
is again feature-major.
"""

import numpy as np

import concourse.bacc as bacc
import concourse.bass as bass
import concourse.mybir as mybir
import concourse.tile as tile
from concourse.bass_utils import run_bass_kernel_spmd

F32 = mybir.dt.float32
AF = mybir.ActivationFunctionType
ALU = mybir.AluOpType

B, NPG, IN_C, HID, INDIM, DEG, KNN, DIL = 512, 100, 100, 256, 128, 10, 4, 2
N = B * NPG
NCORES = 8
GPC = B // NCORES      # graphs per core (64)
GRP = 8                # graphs per inner group (batching unit for DVE ops)

PASSES = ("hc", "dis")
DEBUG = False
# (name, Cin, Fout, has_knn, combine)  combine: 'leaky' | 'add'
CONV_LAYERS = {
    "enc0": (IN_C, HID, "leaky"),
    "enc1": (HID, INDIM, "leaky"),
    "dec0": (INDIM, HID, "add"),
    "dec1": (HID, IN_C, "add"),
}


def _nchunks(c):
    return (c + 127) // 128


def _csz(c, i):
    return min(128, c - i * 128)


class Program:
    """Builds the per-core Bass program (identical on all cores)."""

    def __init__(self, nc, gpc=GPC, grp=GRP):
        self._dbg_grp0 = False
        self.nc = nc
        self.gpc = gpc
        self.grp = grp
        self.ng = gpc // grp
        self.io = {}
        self._declare_io()

    def _in(self, name, shape):
        ap = self.nc.dram_tensor(name, list(shape), F32, kind="ExternalInput").ap()
        self.io[name] = ap
        return ap

    def _out(self, name, shape):
        ap = self.nc.dram_tensor(name, list(shape), F32, kind="ExternalOutput").ap()
        self.io[name] = ap
        return ap

    def _declare_io(self):
        gpc = self.gpc
        self._in("xT", (IN_C, gpc * NPG))
        self._in("m1t", (NPG, gpc * NPG))
        self._in("eyes", (gpc * NPG, NPG))
        self._in("i100", (NPG, NPG))
        self._in("iotar", (NPG, NPG))
        self._in("mhalf", (128, NPG))
        self._in("onesg", (1, gpc))
        for pp in PASSES:
            self._in(f"epsT_{pp}", (INDIM, gpc))
            self._in(f"w_enc0_{pp}", (IN_C, HID))
            self._in(f"b_enc0_{pp}", (128, 2))
            self._in(f"w_enc1_{pp}", (128, 2 * INDIM))
            self._in(f"b_enc1_{pp}", (INDIM, 1))
            self._in(f"w_read_{pp}", (INDIM, 1))
            self._in(f"b_read_{pp}", (NPG, 1))
            self._in(f"w_rev_{pp}", (1, INDIM))
            self._in(f"b_rev_{pp}", (INDIM, 1))
            self._in(f"w_dec0_{pp}", (INDIM, HID))
            self._in(f"b_dec0_{pp}", (128, 2))
            self._in(f"w_dec1_{pp}", (128, 2 * IN_C))
            self._in(f"b_dec1_{pp}", (IN_C, 1))
            self._in(f"w_mu_{pp}", (NPG, INDIM))
            self._in(f"w_var_{pp}", (NPG, INDIM))
            self._in(f"w_dis_{pp}", (NPG, 2))
            self._in(f"w_pred_{pp}", (NPG, 2))
            self._in(f"w_zdec_{pp}", (INDIM, NPG))
            self._in(f"bc_mu_{pp}", (INDIM, 1))
            self._in(f"bch_var_{pp}", (INDIM, 1))
            self._in(f"bc_zdec_{pp}", (NPG, 1))
            self._in(f"br_mu_{pp}", (1, INDIM))
            self._in(f"br_var_{pp}", (1, INDIM))
            self._in(f"br_dis_{pp}", (1, 2))
            self._in(f"br_pred_{pp}", (1, 2))
            self._out(f"aff_{pp}", (gpc * NPG, NPG))
            if DEBUG:
                for ln, (ci, fo, _) in CONV_LAYERS.items():
                    nch = _nchunks(fo)
                    po = min(fo, 128)
                    self._out(f"dbg_x1_{ln}_{pp}", (po, self.grp * nch * NPG))
                    self._out(f"dbg_x2_{ln}_{pp}", (po, self.grp * nch * NPG))
                    self._out(f"dbg_m2t_{ln}_{pp}", (NPG, self.grp * NPG))
                self._out(f"dbg_zT_{pp}", (NPG, gpc))
                self._out(f"dbg_xrev_{pp}", (INDIM, self.grp * NPG))
                self._out(f"dbg_xd0_{pp}", (128, self.grp * 2 * NPG))
                self._out(f"dbg_xd1_{pp}", (IN_C, self.grp * NPG))
            self._out(f"hdis_{pp}", (gpc, 2))
            self._out(f"ypred_{pp}", (gpc, 2))
            self._out(f"mu_{pp}", (gpc, INDIM))
            self._out(f"lv_{pp}", (gpc, INDIM))

    # ---------------- tile program ----------------

    def build(self, tc):
        nc = self.nc
        io = self.io
        grp, ng, gpc = self.grp, self.ng, self.gpc

        import contextlib

        ctx = contextlib.ExitStack()
        self.ctx = ctx
        cpool = ctx.enter_context(tc.tile_pool(name="cpool", bufs=1))
        gpool = ctx.enter_context(tc.tile_pool(name="gpool", bufs=2))
        wpool = ctx.enter_context(tc.tile_pool(name="wpool", bufs=2))
        xpool = ctx.enter_context(tc.tile_pool(name="xpool", bufs=3))
        zpool = ctx.enter_context(tc.tile_pool(name="zpool", bufs=grp + 2))
        spool = ctx.enter_context(tc.tile_pool(name="spool", bufs=2))
        affpool = ctx.enter_context(tc.tile_pool(name="affpool", bufs=4))
        pspool = ctx.enter_context(tc.tile_pool(name="ps", bufs=6, space="PSUM"))
        ps2pool = ctx.enter_context(tc.tile_pool(name="ps2", bufs=2, space="PSUM"))

        self.pools = dict(c=cpool, g=gpool, w=wpool, x=xpool, z=zpool,
                          s=spool, aff=affpool, ps=pspool, ps2=ps2pool)

        # --- persistent constants / params ---
        def load_const(name, shape):
            t = cpool.tile(list(shape), F32, tag=name)
            nc.sync.dma_start(out=t[:], in_=io[name][:])
            return t

        C = {}
        for name in ("i100", "iotar", "mhalf", "onesg"):
            C[name] = load_const(name, io[name].shape)
        for pp in PASSES:
            for name in (f"epsT_{pp}",
                         f"w_enc0_{pp}", f"b_enc0_{pp}", f"w_enc1_{pp}",
                         f"b_enc1_{pp}", f"w_read_{pp}", f"b_read_{pp}",
                         f"w_rev_{pp}", f"b_rev_{pp}", f"w_dec0_{pp}",
                         f"b_dec0_{pp}", f"w_dec1_{pp}", f"b_dec1_{pp}",
                         f"w_mu_{pp}", f"w_var_{pp}", f"w_dis_{pp}",
                         f"w_pred_{pp}", f"w_zdec_{pp}", f"bc_mu_{pp}",
                         f"bch_var_{pp}", f"bc_zdec_{pp}", f"br_mu_{pp}",
                         f"br_var_{pp}", f"br_dis_{pp}", f"br_pred_{pp}"):
                C[name] = load_const(name, io[name].shape)
        self.C = C

        hheads = {pp: cpool.tile([NPG, gpc], F32, tag=f"hh_{pp}", name=f"hh_{pp}")
                  for pp in PASSES}

        # ---- phase A: encoder + readout ----
        for g in range(ng):
            m1t_g = gpool.tile([NPG, grp * NPG], F32, tag="m1t", name="m1t")
            nc.sync.dma_start(out=m1t_g[:],
                              in_=io["m1t"][:, g * grp * NPG:(g + 1) * grp * NPG])
            xt0 = gpool.tile([IN_C, grp * NPG], F32, tag="xt0", name="xt0")
            nc.sync.dma_start(out=xt0[:],
                              in_=io["xT"][:, g * grp * NPG:(g + 1) * grp * NPG])
            for pp in PASSES:
                self._dbg_grp0 = (g == 0)
                cur = dict(tile=xt0, cin=IN_C)
                cur = self.fused_conv(tc, cur, m1t_g, pp, "enc0")
                cur = self.fused_conv(tc, cur, m1t_g, pp, "enc1")
                self.readout(tc, cur, m1t_g, pp, hheads[pp], g)

        # ---- phase B: VAE heads ----
        zT = {}
        for pp in PASSES:
            zT[pp] = self.heads(tc, hheads[pp], pp)

        # ---- phase C: decoder + affinity ----
        for g in range(ng):
            m1t_g = gpool.tile([NPG, grp * NPG], F32, tag="m1t", name="m1t")
            nc.sync.dma_start(out=m1t_g[:],
                              in_=io["m1t"][:, g * grp * NPG:(g + 1) * grp * NPG])
            eyes_g = gpool.tile([NPG, grp * NPG], F32, tag="eyes", name="eyes")
            for j in range(grp):
                gg = g * grp + j
                nc.sync.dma_start(
                    out=eyes_g[:, j * NPG:(j + 1) * NPG],
                    in_=io["eyes"][gg * NPG:(gg + 1) * NPG, :])
            for pp in PASSES:
                self._dbg_grp0 = (g == 0)
                cur = self.rev_conv(tc, zT[pp], m1t_g, pp, g)
                if DEBUG and g == 0:
                    nc.sync.dma_start(out=io[f"dbg_xrev_{pp}"][:],
                                      in_=cur["tile"][:])
                cur = self.fused_conv(tc, cur, m1t_g, pp, "dec0")
                if DEBUG and g == 0:
                    nc.sync.dma_start(out=io[f"dbg_xd0_{pp}"][:],
                                      in_=cur["tile"][:])
                cur = self.fused_conv(tc, cur, m1t_g, pp, "dec1")
                if DEBUG and g == 0:
                    nc.sync.dma_start(out=io[f"dbg_xd1_{pp}"][:],
                                      in_=cur["tile"][:])
                self.affinity(tc, cur, eyes_g, pp, g)

        ctx.close()

    # ----- building blocks -----

    def featmm(self, z_ps, cur, w_sb, g, fo):
        """z_ps[100, fo] = x_g @ W   (accumulate over Cin chunks)."""
        nc = self.nc
        cin = cur["cin"]
        nch = _nchunks(cin)
        for c in range(nch):
            pc = _csz(cin, c)
            lhsT = cur["tile"][0:pc, (g * nch + c) * NPG:(g * nch + c + 1) * NPG]
            rhs = w_sb[0:pc, c * fo:(c + 1) * fo]
            nc.tensor.matmul(z_ps[:], lhsT, rhs,
                             start=(c == 0), stop=(c == nch - 1))

    def prop(self, tc, z_sb, at_ap, out_tile, g, fo, bias, scale):
        """out_T chunks = tanh(scale * (A @ z)^T + bias)."""
        nc = self.nc
        nch = _nchunks(fo)
        for c in range(nch):
            pc = _csz(fo, c)
            pt = self.pools["ps"].tile([pc, NPG], F32, tag="ps", name="ps")
            nc.tensor.matmul(pt[:], z_sb[:, c * 128:c * 128 + pc], at_ap,
                             start=True, stop=True)
            nc.scalar.activation(
                out_tile[0:pc, (g * nch + c) * NPG:(g * nch + c + 1) * NPG],
                pt[:], AF.Tanh, bias=bias[0:pc, c:c + 1], scale=scale)

    def fused_conv(self, tc, cur, m1t_g, pp, lname):
        nc = self.nc
        grp = self.grp
        cin, fo, combine = CONV_LAYERS[lname]
        nch_o = _nchunks(fo)
        po = min(fo, 128)
        w_sb = self.C[f"w_{lname}_{pp}"]
        b_sb = self.C[f"b_{lname}_{pp}"]
        width = grp * nch_o * NPG

        x1t = self.pools["w"].tile([po, width], F32, tag="x1", name="x1")
        x2t = self.pools["w"].tile([po, width], F32, tag="x2", name="x2")
        zs = []
        for g in range(grp):
            z_ps = self.pools["ps2"].tile([NPG, fo], F32, tag="zp", name="zp")
            self.featmm(z_ps, cur, w_sb, g, fo)
            z_sb = self.pools["z"].tile([NPG, fo], F32, tag="z", name="z")
            nc.scalar.activation(z_sb[:], z_ps[:], AF.Copy)
            zs.append(z_sb)
            self.prop(tc, z_sb, m1t_g[:, g * NPG:(g + 1) * NPG],
                      x1t, g, fo, b_sb, 1.0)

        m2t = self.knn(tc, x1t, fo, nch_o, po)

        for g in range(grp):
            self.prop(tc, zs[g], m2t[:, g * NPG:(g + 1) * NPG],
                      x2t, g, fo, b_sb, 0.2)
        if DEBUG and self._dbg_grp0:
            nc.sync.dma_start(out=self.io[f"dbg_x1_{lname}_{pp}"][:], in_=x1t[:])
            nc.sync.dma_start(out=self.io[f"dbg_x2_{lname}_{pp}"][:], in_=x2t[:])
            nc.sync.dma_start(out=self.io[f"dbg_m2t_{lname}_{pp}"][:], in_=m2t[:])

        out = self.pools["x"].tile([po, width], F32, tag="xt", name="xt")
        nc.vector.tensor_tensor(out[:], x1t[:], x2t[:], op=ALU.add)
        if combine == "leaky":
            nc.vector.scalar_tensor_tensor(out[:], out[:], 0.01, out[:],
                                           op0=ALU.mult, op1=ALU.max)
        return dict(tile=out, cin=fo)

    def knn(self, tc, x1t, fo, nch_o, po):
        """Build M2^T+I (unnormalized; 0.2 folded into prop scale) [100,grp*100]."""
        nc = self.nc
        grp = self.grp
        sq = self.pools["w"].tile([po, grp * nch_o * NPG], F32, tag="sq", name="sq")
        nc.vector.tensor_tensor(sq[:], x1t[:], x1t[:], op=ALU.mult)

        s3 = self.pools["s"].tile([NPG, grp, NPG], F32, tag="s3", name="s3")
        mx8 = self.pools["s"].tile([NPG, grp, 8], F32, tag="mx8", name="mx8")
        idx8 = self.pools["s"].tile([NPG, grp, 8], mybir.dt.uint32,
                                    tag="idx8", name="idx8")
        idxf = self.pools["s"].tile([NPG, grp, 8], F32, tag="idxf", name="idxf")
        for g in range(grp):
            sp = self.pools["ps"].tile([NPG, NPG], F32, tag="ps", name="ps")
            for c in range(nch_o):
                pc = _csz(fo, c)
                sl = slice((g * nch_o + c) * NPG, (g * nch_o + c + 1) * NPG)
                nc.tensor.matmul(sp[:], x1t[0:pc, sl], x1t[0:pc, sl],
                                 start=(c == 0), stop=False)
            for c in range(nch_o):
                pc = _csz(fo, c)
                sl = slice((g * nch_o + c) * NPG, (g * nch_o + c + 1) * NPG)
                nc.tensor.matmul(sp[:], self.C["mhalf"][0:pc, :], sq[0:pc, sl],
                                 start=False, stop=(c == nch_o - 1))
            nc.vector.tensor_copy(s3[:, g, :], sp[:])
            nc.vector.max(mx8[:, g, :], s3[:, g, :])
            nc.vector.max_index(idx8[:, g, :], mx8[:, g, :], s3[:, g, :])

        nc.vector.tensor_copy(idxf[:], idx8[:])
        t1 = self.pools["s"].tile([NPG, grp, NPG], F32, tag="t1", name="t1")
        t2 = self.pools["s"].tile([NPG, grp, NPG], F32, tag="t2", name="t2")
        iota3 = self.C["iotar"][:].rearrange("p (a m) -> p a m", a=1)
        for i, j in enumerate((1, 3, 5, 7)):
            dst = t1 if i == 0 else t2
            i_ap, v_ap = bass.broadcast_tensor_aps(iota3, idxf[:, :, j:j + 1])
            nc.vector.tensor_tensor(dst[:], i_ap, v_ap, op=ALU.is_equal)
            if i > 0:
                nc.vector.tensor_tensor(t1[:], t1[:], t2[:], op=ALU.add)

        m2t = self.pools["s"].tile([NPG, grp * NPG], F32, tag="m2t", name="m2t")
        for g in range(grp):
            atp = self.pools["ps"].tile([NPG, NPG], F32, tag="ps", name="ps")
            nc.tensor.transpose(atp[:], t1[:, g, :], self.C["i100"][:])
            nc.vector.tensor_tensor(m2t[:, g * NPG:(g + 1) * NPG],
                                    atp[:], self.C["i100"][:], op=ALU.add)
        return m2t

    def readout(self, tc, cur, m1t_g, pp, hh, g):
        nc = self.nc
        grp = self.grp
        for j in range(grp):
            zp = self.pools["ps"].tile([NPG, 1], F32, tag="ps", name="ps")
            self.featmm(zp, cur, self.C[f"w_read_{pp}"], j, 1)
            z_sb = self.pools["z"].tile([NPG, 1], F32, tag="zr", name="zr")
            nc.scalar.activation(z_sb[:], zp[:], AF.Copy)
            hp = self.pools["ps"].tile([NPG, 1], F32, tag="ps", name="ps")
            nc.tensor.matmul(hp[:], m1t_g[:, j * NPG:(j + 1) * NPG], z_sb[:],
                             start=True, stop=True)
            gg = g * grp + j
            nc.scalar.activation(hh[:, gg:gg + 1], hp[:], AF.Tanh,
                                 bias=self.C[f"b_read_{pp}"][:], scale=1.0)

    def heads(self, tc, hh, pp):
        nc = self.nc
        gpc = self.gpc
        C = self.C
        ps = self.pools["ps"]
        cp = self.pools["c"]
        ones = C["onesg"]

        def head_mm(w, brow, fo):
            p = ps.tile([gpc, fo], F32, tag="ps", name="ps")
            nc.tensor.matmul(p[:], hh[:], w[:], start=True, stop=False)
            nc.tensor.matmul(p[:], ones[:], brow[:], start=False, stop=True)
            return p

        # h_dis
        dp = head_mm(C[f"w_dis_{pp}"], C[f"br_dis_{pp}"], 2)
        dis_sb = cp.tile([gpc, 2], F32, tag=f"dis_{pp}", name=f"dis_{pp}")
        nc.scalar.activation(dis_sb[:], dp[:], AF.Sigmoid)
        nc.sync.dma_start(out=self.io[f"hdis_{pp}"][:], in_=dis_sb[:])
        # y_pred
        ppred = head_mm(C[f"w_pred_{pp}"], C[f"br_pred_{pp}"], 2)
        e_sb = cp.tile([gpc, 2], F32, tag=f"pe_{pp}", name=f"pe_{pp}")
        nc.scalar.activation(e_sb[:], ppred[:], AF.Exp)
        ssum = cp.tile([gpc, 1], F32, tag=f"ps_{pp}", name=f"ps_{pp}")
        nc.vector.tensor_reduce(ssum[:], e_sb[:], axis=mybir.AxisListType.X,
                                op=ALU.add)
        rinv = cp.tile([gpc, 1], F32, tag=f"pr_{pp}", name=f"pr_{pp}")
        nc.vector.reciprocal(rinv[:], ssum[:])
        yp_sb = cp.tile([gpc, 2], F32, tag=f"yp_{pp}", name=f"yp_{pp}")
        nc.vector.tensor_scalar(yp_sb[:], e_sb[:], rinv[:], None, op0=ALU.mult)
        nc.sync.dma_start(out=self.io[f"ypred_{pp}"][:], in_=yp_sb[:])
        # mu / log_var (row-major outputs)
        for wname, brow, oname in ((f"w_mu_{pp}", f"br_mu_{pp}", f"mu_{pp}"),
                                   (f"w_var_{pp}", f"br_var_{pp}", f"lv_{pp}")):
            mp = head_mm(C[wname], C[brow], INDIM)
            m_sb = cp.tile([gpc, INDIM], F32, tag=f"o_{oname}", name=f"o_{oname}")
            nc.vector.tensor_copy(m_sb[:], mp[:])
            nc.sync.dma_start(out=self.io[oname][:], in_=m_sb[:])
        # muT / stdT -> zzT -> zT
        mtp = ps.tile([INDIM, gpc], F32, tag="ps", name="ps")
        nc.tensor.matmul(mtp[:], C[f"w_mu_{pp}"][:], hh[:], start=True, stop=True)
        muT = cp.tile([INDIM, gpc], F32, tag=f"muT_{pp}", name=f"muT_{pp}")
        nc.scalar.activation(muT[:], mtp[:], AF.Identity,
                             bias=C[f"bc_mu_{pp}"][:], scale=1.0)
        vtp = ps.tile([INDIM, gpc], F32, tag="ps", name="ps")
        nc.tensor.matmul(vtp[:], C[f"w_var_{pp}"][:], hh[:], start=True, stop=True)
        stdT = cp.tile([INDIM, gpc], F32, tag=f"stdT_{pp}", name=f"stdT_{pp}")
        nc.scalar.activation(stdT[:], vtp[:], AF.Exp,
                             bias=C[f"bch_var_{pp}"][:], scale=0.5)
        zzT = cp.tile([INDIM, gpc], F32, tag=f"zzT_{pp}", name=f"zzT_{pp}")
        nc.vector.tensor_tensor(zzT[:], C[f"epsT_{pp}"][:], stdT[:], op=ALU.mult)
        nc.vector.tensor_tensor(zzT[:], zzT[:], muT[:], op=ALU.add)
        ztp = ps.tile([NPG, gpc], F32, tag="ps", name="ps")
        nc.tensor.matmul(ztp[:], C[f"w_zdec_{pp}"][:], zzT[:], start=True, stop=True)
        zT = cp.tile([NPG, gpc], F32, tag=f"zT_{pp}", name=f"zT_{pp}")
        nc.scalar.activation(zT[:], ztp[:], AF.Identity,
                             bias=C[f"bc_zdec_{pp}"][:], scale=1.0)
        if DEBUG:
            nc.sync.dma_start(out=self.io[f"dbg_zT_{pp}"][:], in_=zT[:])
        return zT

    def rev_conv(self, tc, zT, m1t_g, pp, g):
        nc = self.nc
        grp = self.grp
        xrev = self.pools["x"].tile([INDIM, grp * NPG], F32, tag="xt", name="xt")
        for j in range(grp):
            gg = g * grp + j
            mzp = self.pools["ps"].tile([1, NPG], F32, tag="ps", name="ps")
            nc.tensor.matmul(mzp[:], zT[:, gg:gg + 1],
                             m1t_g[:, j * NPG:(j + 1) * NPG],
                             start=True, stop=True)
            mz_sb = self.pools["z"].tile([1, NPG], F32, tag="mz", name="mz")
            nc.scalar.activation(mz_sb[:], mzp[:], AF.Copy)
            xrp = self.pools["ps"].tile([INDIM, NPG], F32, tag="ps", name="ps")
            nc.tensor.matmul(xrp[:], self.C[f"w_rev_{pp}"][:], mz_sb[:],
                             start=True, stop=True)
            nc.scalar.activation(xrev[:, j * NPG:(j + 1) * NPG], xrp[:],
                                 AF.Tanh, bias=self.C[f"b_rev_{pp}"][:],
                                 scale=1.0)
        return dict(tile=xrev, cin=INDIM)

    def affinity(self, tc, cur, eyes_g, pp, g):
        nc = self.nc
        grp = self.grp
        for j in range(grp):
            gg = g * grp + j
            afp = self.pools["ps"].tile([NPG, NPG], F32, tag="ps", name="ps")
            sl = cur["tile"][0:IN_C, j * NPG:(j + 1) * NPG]
            nc.tensor.matmul(afp[:], sl, sl, start=True, stop=True)
            t = self.pools["aff"].tile([NPG, NPG], F32, tag="afft", name="afft")
            nc.vector.scalar_tensor_tensor(
                t[:], eyes_g[:, j * NPG:(j + 1) * NPG], -1.0, afp[:],
                op0=ALU.mult, op1=ALU.mult)
            o = self.pools["aff"].tile([NPG, NPG], F32, tag="affo", name="affo")
            nc.vector.tensor_tensor(o[:], afp[:], t[:], op=ALU.add)
            nc.sync.dma_start(
                out=self.io[f"aff_{pp}"][gg * NPG:(gg + 1) * NPG, :], in_=o[:])


# ---------------- host side ----------------

def _build_m1t(src, dst):
    g = dst // NPG
    d_loc = (dst % NPG).astype(np.int64)
    s_loc = (src % NPG).astype(np.int64)
    flat = g.astype(np.int64) * NPG * NPG + d_loc * NPG + s_loc
    cnt = np.bincount(flat, minlength=B * NPG * NPG).astype(np.float32)
    cnt = cnt.reshape(B, NPG, NPG)
    deg = cnt.sum(axis=2) + 1.0
    dinv = (1.0 / np.sqrt(np.maximum(deg, 1.0))).astype(np.float32)
    cnt += np.eye(NPG, dtype=np.float32)[None]
    m1 = dinv[:, :, None] * cnt * dinv[:, None, :]
    return np.ascontiguousarray(m1.transpose(0, 2, 1))  # [g, s, d]


def _eps_arrays():
    import jax
    with jax.default_device(jax.devices("cpu")[0]):
        k1, k2 = jax.random.split(jax.random.key(42))
        e1 = np.asarray(jax.random.normal(k1, (B, INDIM), np.float32))
        e2 = np.asarray(jax.random.normal(k2, (B, INDIM), np.float32))
    return e1, e2


def _pack_params(p):
    """Host-side packing of one param set into the kernel's tile layouts."""
    out = {}
    f32 = lambda a: np.ascontiguousarray(np.asarray(a, np.float32))

    def col(b, p128):
        v = np.zeros((p128, 1), np.float32)
        v[:b.shape[0], 0] = b
        return v

    def chunkcols(b, nch):
        v = np.zeros((128, nch), np.float32)
        for c in range(nch):
            pc = _csz(b.shape[0], c)
            v[:pc, c] = b[c * 128:c * 128 + pc]
        return v

    def kchunks(w, fo):
        # [Cin, fo] -> [128, nch*fo] with chunk c = rows c*128..+pc
        cin = w.shape[0]
        nch = _nchunks(cin)
        v = np.zeros((128, nch * fo), np.float32)
        for c in range(nch):
            pc = _csz(cin, c)
            v[:pc, c * fo:(c + 1) * fo] = w[c * 128:c * 128 + pc]
        return v

    out["w_enc0"] = f32(p["enc0_W"])                      # [100, 256]
    out["b_enc0"] = chunkcols(f32(p["enc0_b"]), 2)
    out["w_enc1"] = kchunks(f32(p["enc1_W"]), INDIM)      # [128, 256]
    out["b_enc1"] = col(f32(p["enc1_b"]), INDIM)
    out["w_read"] = f32(p["read_W"])                      # [128, 1]
    out["b_read"] = np.full((NPG, 1), np.float32(np.asarray(p["read_b"])[0]))
    out["w_rev"] = f32(p["rev_W"])                        # [1, 128]
    out["b_rev"] = col(f32(p["rev_b"]), INDIM)
    out["w_dec0"] = f32(p["dec0_W"])                      # [128, 256]
    out["b_dec0"] = chunkcols(f32(p["dec0_b"]), 2)
    out["w_dec1"] = kchunks(f32(p["dec1_W"]), IN_C)       # [128, 200]
    out["b_dec1"] = col(f32(p["dec1_b"]), IN_C)
    out["w_mu"] = f32(p["mu_W"])
    out["w_var"] = f32(p["var_W"])
    out["w_dis"] = f32(p["dis_W"])
    out["w_pred"] = f32(p["pred_W"])
    out["w_zdec"] = f32(p["zdec_W"])
    out["bc_mu"] = col(f32(p["mu_b"]), INDIM)
    out["bch_var"] = col(0.5 * f32(p["var_b"]), INDIM)
    out["bc_zdec"] = col(f32(p["zdec_b"]), NPG)
    out["br_mu"] = f32(p["mu_b"])[None, :]
    out["br_var"] = f32(p["var_b"])[None, :]
    out["br_dis"] = f32(p["dis_b"])[None, :]
    out["br_pred"] = f32(p["pred_b"])[None, :]
    return out


_PROGRAM_CACHE = {}


def _get_program(gpc=GPC, grp=GRP):
    key = (gpc, grp)
    if key not in _PROGRAM_CACHE:
        nc = bacc.Bacc("TRN2", target_bir_lowering=False, debug=False)
        prog = Program(nc, gpc=gpc, grp=grp)
        with tile.TileContext(nc) as tc:
            prog.build(tc)
        nc.compile()
        _PROGRAM_CACHE[key] = prog
    return _PROGRAM_CACHE[key]


def run(x, edge_index, eyes, params_hc, params_dis, gpc=GPC, grp=GRP,
        n_cores=NCORES, trace=False):
    """Shard, execute on n_cores, gather. Returns the 10-tuple + perf info."""
    x = np.asarray(x, np.float32)
    src = np.asarray(edge_index[0])
    dst = np.asarray(edge_index[1])
    eyes = np.asarray(eyes, np.float32)
    assert np.array_equal(src // NPG, dst // NPG), "edges must be within-graph"

    m1t = _build_m1t(src, dst)  # [B, 100, 100] (s, d)
    eps_hc, eps_dis = _eps_arrays()
    packed = {"hc": _pack_params(params_hc), "dis": _pack_params(params_dis)}

    consts = {
        "i100": np.eye(NPG, dtype=np.float32),
        "iotar": np.broadcast_to(np.arange(NPG, dtype=np.float32),
                                 (NPG, NPG)).copy(),
        "mhalf": np.full((128, NPG), -0.5, np.float32),
        "onesg": np.ones((1, gpc), np.float32),
    }

    prog = _get_program(gpc, grp)
    in_maps = []
    for c in range(n_cores):
        gsl = slice(c * gpc, (c + 1) * gpc)
        nsl = slice(c * gpc * NPG, (c + 1) * gpc * NPG)
        xc = x[nsl].reshape(gpc, NPG, IN_C)
        m = dict(consts)
        m["xT"] = np.ascontiguousarray(
            xc.transpose(2, 0, 1).reshape(IN_C, gpc * NPG))
        m["m1t"] = np.ascontiguousarray(
            m1t[gsl].transpose(1, 0, 2).reshape(NPG, gpc * NPG))
        m["eyes"] = np.ascontiguousarray(eyes[nsl])
        for pp, eps in (("hc", eps_hc), ("dis", eps_dis)):
            m[f"epsT_{pp}"] = np.ascontiguousarray(eps[gsl].T)
            for k, v in packed[pp].items():
                m[f"{k}_{pp}"] = v
        in_maps.append(m)

    res = run_bass_kernel_spmd(prog.nc, in_maps, core_ids=list(range(n_cores)),
                               trace=trace)
    outs = []
    for pp in PASSES:
        for name in (f"aff_{pp}", f"hdis_{pp}", f"ypred_{pp}", f"mu_{pp}",
                     f"lv_{pp}"):
            outs.append(np.concatenate(
                [res.results[c][name] for c in range(n_cores)], axis=0))
    return tuple(outs), res


def kernel(x, edge_index, edge_attr, pcd, eyes, batch, params_hc, params_dis):
    outs, _ = run(x, edge_index, eyes, params_hc, params_dis)
    return outs
